# revision 34
# baseline (speedup 1.0000x reference)
"""Trainium2 Bass kernel for nn_MultiHeadAttention_56676388438432.

Reference math (all H=16 heads share identical weights, so they collapse):
    q = query @ Wq; k = key @ Wk; v = value @ Wv          (full-width, [B,S,D])
    qh = q @ wq_h + bq_h                                   ([B,S,64])
    scores = qh @ kh^T / 8, causal mask, softmax
    out_h = attn @ vh
    out = tile(out_h, 16) @ Wo

Algebraic collapse (exact):
    qh = query @ (Wq @ wq_h) + bq_h     -> combined Aq [1024, 64]
    out = out_h @ sum_h Wo[64h:64h+64]  -> combined WoS [64, 1024]

Sharding: 8 cores = 4 batches x 2 balanced causal halves of the query rows
(half 0: rows 0:512 and 1536:2048; half 1: rows 512:1536). Each core
redundantly computes kh/vh for its batch. Uniform SPMD program; per-core
causal masking is encoded in host-baked additive mask bands.
"""

import sys

sys.path.insert(0, "/opt/trn_rl_repo")

from contextlib import ExitStack

import numpy as np

import concourse.bass as bass
import concourse.tile as tile
from concourse import bacc
from concourse import mybir
from concourse.bass_utils import run_bass_kernel_spmd

B, S, D, H, HD = 4, 2048, 1024, 16, 64
P = 128
NCORES = 8
NEG = -1.0e9

F32 = mybir.dt.float32
F32R = mybir.dt.float32r

# window setup: every core has 2 q-windows of 512 rows.
#   core c: b = c//2, h = c%2
#   Q0(w=0) = 512*h          (slots: 8)
#   Q0(w=1) = 1536 - 512*h   (slots: 16)
W_SLOTS = (8, 16)
BAND = 1408  # mask band width: 128*7 + 512

# packed f32 const tensor "cf": [biases(3) | ones(16)]
CF_B, CF_ONES = 0, 3
CF_N = 19
# bf16 mask-band tensor "mu": [mu0 | mu1], values only 0 / -1e9
# packed f32r const tensor "cr": [ident(128) | wq(512) | wk(512) | wv(512) | wo(1024)]
CR_ID, CR_WQ, CR_WK, CR_WV, CR_WO = 0, 128, 640, 1152, 1664
CR_N = 2688


def _q0(h, w):
    return 512 * h if w == 0 else 1536 - 512 * h


def _r(ap, dt):
    return ap.bitcast(dt)


def _emit(tc, io):
    """Emit the per-core program. io: dict of DRAM APs."""
    nc = tc.nc
    ctx = ExitStack()
    with ctx:
        # ---------------- pools ----------------
        const = ctx.enter_context(tc.tile_pool(name="const", bufs=1))
        xpool = ctx.enter_context(tc.tile_pool(name="x", bufs=12))
        xtp = ctx.enter_context(tc.tile_pool(name="xT", bufs=12))
        vtp = ctx.enter_context(tc.tile_pool(name="vhT", bufs=2))
        atp = ctx.enter_context(tc.tile_pool(name="attn", bufs=4))
        ohp = ctx.enter_context(tc.tile_pool(name="outh", bufs=2))
        opool = ctx.enter_context(tc.tile_pool(name="o", bufs=4))
        esp = ctx.enter_context(tc.tile_pool(name="es", bufs=2))

        ps_t = ctx.enter_context(tc.tile_pool(name="ps_t", bufs=6, space="PSUM"))
        ps_o = ctx.enter_context(tc.tile_pool(name="ps_o", bufs=2, space="PSUM"))

        # ---------------- constants -------------------------------------
        # ident (64KB) alone first so transposes can start ASAP; the rest of
        # the weight pack and the mask bands follow the first x loads.
        cr = const.tile([P, CR_N], F32R, tag="cr")
        nc.sync.dma_start(cr[:, 0:P], io["cr"][:, 0:P].bitcast(F32R))
        cf = const.tile([P, CF_N], F32, tag="cf")
        nc.sync.dma_start(cf[:], io["cf"][:])
        mub = const.tile([P, 2 * BAND], mybir.dt.bfloat16, tag="mub")

        ident = cr[:, CR_ID : CR_ID + P].bitcast(F32)
        ident_r = cr[:, CR_ID : CR_ID + P]
        wqkv = {
            "wq": cr[:, CR_WQ : CR_WQ + 512],
            "wk": cr[:, CR_WK : CR_WK + 512],
            "wv": cr[:, CR_WV : CR_WV + 512],
        }
        wo = cr[0:HD, CR_WO : CR_WO + D]
        biases = {
            "bq": cf[0:HD, CF_B : CF_B + 1],
            "bk": cf[0:HD, CF_B + 1 : CF_B + 2],
            "bv": cf[0:HD, CF_B + 2 : CF_B + 3],
        }

        khT = const.tile([HD, S], F32R, tag="khT")
        qhT = const.tile([HD, 1024], F32R, tag="qhT")
        vh = const.tile([P, 16 * 65], F32R, tag="vh")  # [:, 65c:65c+64] + ones col
        recip = const.tile([P, 8], F32, tag="recip")
        # ones column at [:, 65c+64] via strided DMA from packed input
        nc.sync.dma_start(
            vh[:].rearrange("p (c e) -> p c e", e=65)[:, :, 64:65],
            io["cf"][:, CF_ONES : CF_ONES + 16].bitcast(F32R),
        )

        copy_flip = [0]
        copy_mode = ["alt"]

        def psum2sb(dst, src):
            # alternate PSUM->SBUF copies between ScalarE and VectorE
            if copy_mode[0] == "act" or copy_flip[0] % 2 == 0:
                nc.scalar.copy(dst, src)
            else:
                nc.vector.tensor_copy(dst, src)
            copy_flip[0] += 1

        def load_block(x_dram, row0):
            """One 1MB DMA covering rows [row0, row0+256) -> [P, 2*D] tile."""
            xt = xpool.tile([P, 2 * D], F32, tag="x")
            nc.sync.dma_start(
                xt[:].rearrange("p (t d) -> p t d", t=2),
                x_dram[row0 : row0 + 256, :].rearrange("(t p) d -> p t d", p=P),
            )
            return xt

        def proj_block(xts, w_sb, finish):
            """Transpose + project a 512-row block given two 256-row tiles."""
            xT = []
            for j in range(8):
                pst = ps_t.tile([P, 512], F32, tag="ps_t")
                for t in range(4):
                    xt = xts[t // 2]
                    tt = t % 2
                    nc.tensor.transpose(
                        pst[:, t * P : (t + 1) * P],
                        xt[:, tt * D + j * P : tt * D + (j + 1) * P],
                        ident,
                    )
                sb = xtp.tile([P, 512], F32R, tag="xT")
                psum2sb(sb[:], pst[:])
                xT.append(sb)
            psp_full = ps_t.tile([P, 512], F32, tag="ps_t")
            psp = psp_full[0:HD]
            for j in range(8):
                nc.tensor.matmul(
                    psp,
                    _r(wqkv[w_sb][:, j * HD : (j + 1) * HD], F32R),
                    _r(xT[j][:], F32R),
                    start=(j == 0),
                    stop=(j == 7),
                )
            finish(psp)

        # ---------------- projections ----------------
        def fin_q_mk(w):
            def fin_q(psp):
                nc.scalar.activation(
                    qhT[:, w * 512 : (w + 1) * 512],
                    psp,
                    mybir.ActivationFunctionType.Identity,
                    bias=biases["bq"],
                    scale=1.0,
                )
            return fin_q

        def fin_k_mk(blk):
            def fin_k(psp):
                nc.scalar.activation(
                    khT[:, blk * 512 : (blk + 1) * 512],
                    psp,
                    mybir.ActivationFunctionType.Identity,
                    bias=biases["bk"],
                    scale=1.0,
                )
            return fin_k

        def fin_v_mk(blk):
            def fin_v(psp):
                vt = vtp.tile([HD, 512], F32R, tag="vhT")
                nc.scalar.activation(
                    vt[:],
                    psp,
                    mybir.ActivationFunctionType.Identity,
                    bias=biases["bv"],
                    scale=1.0,
                )
                for t in range(4):
                    c = blk * 4 + t
                    psv = ps_t.tile([P, 512], F32, tag="ps_t")
                    nc.tensor.transpose(
                        psv[:, 0:HD].bitcast(F32R),
                        vt[:, t * P : (t + 1) * P],
                        ident_r[:HD, :HD],
                    )
                    psum2sb(vh[:, c * 65 : c * 65 + HD], psv[:, 0:HD])
            return fin_v

        mu0 = mub[:, 0:BAND]
        mu1 = mub[:, BAND : 2 * BAND]

        # ------------- fused projection + attention schedule -------------
        # Emission order matches DMA arrival so engine FIFOs never head-of-
        # line block: q projs, then per kv-block-pair: k/v projs followed by
        # the attention slots those chunks unlock.
        psos = []
        for w in range(2):
            pso = ps_o.tile([65, 512], F32, tag="ps_o")
            psos.append(pso)

        def slot(w, j):
            nslots = W_SLOTS[w]
            muw = (mu0, mu1)[w]
            pss = ps_t.tile([P, 512], F32, tag="ps_t")
            nc.tensor.matmul(
                pss[:],
                _r(khT[:, j * P : (j + 1) * P], F32R),
                _r(qhT[:, w * 512 : (w + 1) * 512], F32R),
                start=True,
                stop=True,
            )
            if w == 0 or j >= 8:
                toff = P * (nslots - 1 - j)
                nc.vector.tensor_add(pss[:], pss[:], muw[:, toff : toff + 512])
            attn = atp.tile([P, 512], F32R, tag="attn")
            nc.scalar.activation(attn[:], pss[:], mybir.ActivationFunctionType.Exp)
            nc.tensor.matmul(
                psos[w][:],
                _r(vh[:, j * 65 : j * 65 + 65], F32R),
                _r(attn[:], F32R),
                start=(j == 0),
                stop=(j == nslots - 1),
            )

        def finish_window(w):
            oh = ohp.tile([65, 512], F32R, tag="outh")
            nc.vector.tensor_copy(oh[:], psos[w][:])
            # expsum row -> per-partition scalars via 4 tiny PE transposes
            pse = ps_t.tile([P, 512], F32, tag="ps_t")
            for t in range(4):
                nc.tensor.transpose(
                    pse[:, t : t + 1],
                    oh[64:65, t * P : (t + 1) * P].bitcast(F32),
                    ident[64:65, 64:65],
                )
            nc.vector.reciprocal(recip[:, w * 4 : w * 4 + 4], pse[:, 0:4])
            for t in range(4):
                ot = opool.tile([P, D], F32, tag="o")
                for nh in range(2):
                    psf = ps_t.tile([P, 512], F32, tag="ps_t")
                    nc.tensor.matmul(
                        psf[:],
                        _r(oh[0:HD, t * P : (t + 1) * P], F32R),
                        _r(wo[:, nh * 512 : (nh + 1) * 512], F32R),
                        start=True,
                        stop=True,
                    )
                    dst = ot[:, nh * 512 : (nh + 1) * 512]
                    rs = recip[:, w * 4 + t : w * 4 + t + 1]
                    if (t + nh) % 2 == 0:
                        nc.vector.tensor_scalar_mul(dst, psf[:], rs)
                    else:
                        nc.scalar.activation(
                            dst, psf[:], mybir.ActivationFunctionType.Copy, scale=rs
                        )
                eng = nc.gpsimd if w == 0 else nc.sync
                eng.dma_start(
                    io["out"][w * 512 + t * P : w * 512 + (t + 1) * P, :],
                    ot[:],
                )

        loads = []
        for blk2 in range(2):
            loads.append((io["xq"], blk2 * 256))
        for blk2 in range(2, 4):
            loads.append((io["xq"], blk2 * 256))
        loads2 = []
        for blk in range(4):
            for half in range(2):
                loads2.append((io["xk"], blk * 512 + half * 256))
            for half in range(2):
                loads2.append((io["xv"], blk * 512 + half * 256))
        loads = loads[:2] + loads[2:] + loads2
        tiles = [load_block(d, r) for d, r in loads[:2]]
        # weight pack (needed by first projection) and W0 mask band
        nc.sync.dma_start(cr[:, P:CR_N], io["cr"][:, P:CR_N].bitcast(F32R))
        nc.sync.dma_start(mub[:, 0:BAND], io["mu"][0])
        tiles += [load_block(d, r) for d, r in loads[2:6]]

        def maybe_load(n=1):
            for _ in range(n):
                if len(tiles) < len(loads):
                    d2, r2 = loads[len(tiles)]
                    tiles.append(load_block(d2, r2))

        proj_block(tiles[0:2], "wq", fin_q_mk(0))
        maybe_load(2)
        proj_block(tiles[2:4], "wq", fin_q_mk(1))
        maybe_load(2)
        for pair in range(4):
            i0 = 4 + 4 * pair
            maybe_load(2)
            proj_block(tiles[i0 : i0 + 2], "wk", fin_k_mk(pair))
            maybe_load(2)
            if pair == 1:
                # second mask band, needed by W1 slots 8..15
                nc.sync.dma_start(mub[:, BAND : 2 * BAND], io["mu"][1])
            proj_block(tiles[i0 + 2 : i0 + 4], "wv", fin_v_mk(pair))
            for j in range(4 * pair, 4 * pair + 4):
                if pair < 2:
                    slot(0, j)
                slot(1, j)
            if pair == 1:
                finish_window(0)
        finish_window(1)


_CACHE = {}


def _build():
    if "nc" in _CACHE:
        return _CACHE["nc"]
    nc = bacc.Bacc("TRN2", target_bir_lowering=False, debug=False, num_devices=NCORES)
    io = {}
    io["xq"] = nc.dram_tensor("xq", [1024, D], F32, kind="ExternalInput").ap()
    io["xk"] = nc.dram_tensor("xk", [S, D], F32, kind="ExternalInput").ap()
    io["xv"] = nc.dram_tensor("xv", [S, D], F32, kind="ExternalInput").ap()
    io["cf"] = nc.dram_tensor("cf", [P, CF_N], F32, kind="ExternalInput").ap()
    io["mu"] = nc.dram_tensor(
        "mu", [2, P, BAND], mybir.dt.bfloat16, kind="ExternalInput"
    ).ap()
    io["cr"] = nc.dram_tensor("cr", [P, CR_N], F32, kind="ExternalInput").ap()
    io["out"] = nc.dram_tensor("out", [1024, D], F32, kind="ExternalOutput").ap()
    with tile.TileContext(nc) as tc:
        _emit(tc, io)
    nc.compile()
    _CACHE["nc"] = nc
    return nc


def _host_prep(query, key, value, mask, Wq, Wk, Wv, wq_h, bq_h, wk_h, bk_h, wv_h,
               bv_h, Wo):
    """Combine weights on host (exact algebra, float64 accumulate)."""
    Aq = (np.asarray(Wq, np.float64) @ np.asarray(wq_h, np.float64) / 8.0).astype(
        np.float32
    )
    Ak = (np.asarray(Wk, np.float64) @ np.asarray(wk_h, np.float64)).astype(np.float32)
    Av = (np.asarray(Wv, np.float64) @ np.asarray(wv_h, np.float64)).astype(np.float32)
    bq = (np.asarray(bq_h, np.float64) / 8.0).astype(np.float32)
    bk = np.asarray(bk_h, np.float32)
    bv = np.asarray(bv_h, np.float32)
    WoS = (
        np.asarray(Wo, np.float64).reshape(H, HD, D).sum(axis=0).astype(np.float32)
    )
    return Aq, Ak, Av, bq, bk, bv, WoS


def _pack_w(A):
    """[1024, 64] -> [128, 512] partition-packed layout."""
    return np.ascontiguousarray(
        A.reshape(8, P, HD).transpose(1, 0, 2).reshape(P, 512)
    )


def _mk_cf(h):
    """Per-core packed f32 consts: biases, ones."""
    cf = np.zeros((P, CF_N), np.float32)
    cf[:, CF_ONES : CF_ONES + 16] = 1.0
    return cf


def _mk_mu(h):
    """Per-core bf16 mask bands (values 0 / -1e9 exactly representable-ish)."""
    import ml_dtypes

    kk = np.arange(BAND)[None, :]
    pp = np.arange(P)[:, None]

    def band(X):
        return np.where(pp > kk - X, np.float32(NEG), np.float32(0.0))

    mu = np.stack([band(896 - _q0(h, 0)), band(1920 - _q0(h, 1))], axis=0)
    return mu.astype(ml_dtypes.bfloat16)


def _numpy_fallback(query, key, value, mask, Wq, Wk, Wv, wq_h, bq_h, wk_h, bk_h,
                    wv_h, bv_h, Wo):
    q = query @ Wq
    k = key @ Wk
    v = value @ Wv
    qh = q @ wq_h + bq_h
    kh = k @ wk_h + bk_h
    vh = v @ wv_h + bv_h
    scores = np.einsum("bsh,bth->bst", qh, kh) / np.sqrt(np.float32(HD))
    scores = np.where(mask, np.float32(-1e9), scores)
    scores = scores - scores.max(axis=-1, keepdims=True)
    e = np.exp(scores)
    attn = e / e.sum(axis=-1, keepdims=True)
    out_h = np.einsum("bst,bth->bsh", attn, vh)
    out = np.tile(out_h, (1, 1, H))
    return (out @ Wo).astype(np.float32)


def kernel(**inputs):
    inputs = {k: np.asarray(v) for k, v in inputs.items()}
    mask = inputs["mask"]
    causal = np.array_equal(mask, np.triu(np.ones((S, S), bool), k=1))
    if not causal:
        return _numpy_fallback(**inputs)

    query, key, value = inputs["query"], inputs["key"], inputs["value"]
    Aq, Ak, Av, bq, bk, bv, WoS = _host_prep(**inputs)

    cr = np.zeros((P, CR_N), np.float32)
    cr[:, CR_ID : CR_ID + P] = np.eye(P, dtype=np.float32)
    cr[:, CR_WQ : CR_WQ + 512] = _pack_w(Aq)
    cr[:, CR_WK : CR_WK + 512] = _pack_w(Ak)
    cr[:, CR_WV : CR_WV + 512] = _pack_w(Av)
    cr[0:HD, CR_WO : CR_WO + D] = WoS

    nc = _build()
    in_maps = []
    for c in range(NCORES):
        b, h = c // 2, c % 2
        xq = np.concatenate(
            [query[b, _q0(h, 0) : _q0(h, 0) + 512], query[b, _q0(h, 1) : _q0(h, 1) + 512]],
            axis=0,
        )
        cf = _mk_cf(h)
        cf[0:HD, CF_B] = bq
        cf[0:HD, CF_B + 1] = bk
        cf[0:HD, CF_B + 2] = bv
        in_maps.append(
            {
                "xq": np.ascontiguousarray(xq, np.float32),
                "xk": np.ascontiguousarray(key[b], np.float32),
                "xv": np.ascontiguousarray(value[b], np.float32),
                "cf": cf,
                "cr": cr,
                "mu": _mk_mu(h),
            }
        )

    res = run_bass_kernel_spmd(nc, in_maps, list(range(NCORES)))
    out = np.empty((B, S, D), np.float32)
    for c in range(NCORES):
        b, h = c // 2, c % 2
        co = res.results[c]["out"]
        out[b, _q0(h, 0) : _q0(h, 0) + 512] = co[0:512]
        out[b, _q0(h, 1) : _q0(h, 1) + 512] = co[512:1024]
    return out


if __name__ == "__main__":
    nc = _build()
    print("build ok")
